# revision 4
# baseline (speedup 1.0000x reference)
"""AxialUNet forward on 8 Trainium2 NeuronCores.

Sharding: H-slabs across 8 cores (each core holds H/8 rows of both batch
elements). W-axis attention, conv1x1, maxpool are row-local. H-axis
attention runs after an all_to_all transpose into W-slab layout (full-H
columns local), then transposes back. BatchNorm is sync-BN via psum of
per-core (sum, sumsq). Bilinear upsample all_gathers the (small)
pre-upsample tensor so cross-slab halo rows are available, then keeps own
output rows. Each UNet block is its own jitted shard_map program (much
faster neuron compiles than one monolithic program).
"""
import os
import time
import numpy as np

HEADS = 2
BN_EPS = 1e-5
NCORES = 8

_CACHE = {}
_VERBOSE = os.environ.get("KERNEL_VERBOSE", "0") == "1"


def _log(msg):
    if _VERBOSE:
        print(f"[kernel {time.strftime('%H:%M:%S')}] {msg}", flush=True)


# ---------------- shared math (runs inside shard_map, one core) -----------

def _attend(t, p):
    import jax
    import jax.numpy as jnp
    hid = p['Wq'].shape[1]
    e = hid // HEADS
    q = t @ p['Wq']
    k, v = jnp.split(t @ p['Wkv'], 2, axis=-1)
    sh = lambda z: z.reshape(z.shape[:-1] + (HEADS, e))
    q, k, v = sh(q), sh(k), sh(v)
    dots = jnp.einsum('baihe,bajhe->bahij', q, k) * (e ** -0.5)
    attn = jax.nn.softmax(dots, axis=-1)
    out = jnp.einsum('bahij,bajhe->baihe', attn, v)
    out = out.reshape(out.shape[:-2] + (hid,))
    return out @ p['Wo'] + p['bo']


def _axial_attention_sharded(x, p, axis_name):
    import jax
    import jax.numpy as jnp
    xt = jnp.transpose(x, (0, 2, 3, 1))              # (B, Hl, W, C)
    ow = _attend(xt, p['w'])                         # local: attend over W
    xw = jax.lax.all_to_all(xt, axis_name, split_axis=2, concat_axis=1,
                            tiled=True)              # (B, H, Wl, C)
    xh = jnp.swapaxes(xw, 1, 2)                      # (B, Wl, H, C)
    ohw = jnp.swapaxes(_attend(xh, p['h']), 1, 2)    # (B, H, Wl, C)
    oh = jax.lax.all_to_all(ohw, axis_name, split_axis=1, concat_axis=2,
                            tiled=True)              # (B, Hl, W, C)
    return jnp.transpose(oh + ow, (0, 3, 1, 2))      # (B, C, Hl, W)


def _axial_attention_local(x, p):
    import jax.numpy as jnp
    xc = jnp.transpose(x, (0, 2, 3, 1))
    xh = jnp.swapaxes(xc, 1, 2)
    oh = jnp.swapaxes(_attend(xh, p['h']), 1, 2)
    ow = _attend(xc, p['w'])
    return jnp.transpose(oh + ow, (0, 3, 1, 2))


def _conv1x1(x, w):
    import jax.numpy as jnp
    return jnp.einsum('bchw,oc->bohw', x, w)


def _bn_sharded(x, g, b, axis_name):
    import jax
    import jax.numpy as jnp
    n_local = x.shape[0] * x.shape[2] * x.shape[3]
    s1 = jax.lax.psum(jnp.sum(x, axis=(0, 2, 3)), axis_name)
    s2 = jax.lax.psum(jnp.sum(x * x, axis=(0, 2, 3)), axis_name)
    n = n_local * NCORES
    mean = s1 / n
    var = s2 / n - mean * mean
    xn = (x - mean[None, :, None, None]) * jax.lax.rsqrt(
        var[None, :, None, None] + BN_EPS)
    return xn * g[None, :, None, None] + b[None, :, None, None]


def _bn_local(x, g, b):
    import jax
    import jax.numpy as jnp
    mean = jnp.mean(x, axis=(0, 2, 3), keepdims=True)
    var = jnp.var(x, axis=(0, 2, 3), keepdims=True)
    xn = (x - mean) * jax.lax.rsqrt(var + BN_EPS)
    return xn * g[None, :, None, None] + b[None, :, None, None]


def _maxpool2(x):
    b, c, h, w = x.shape
    return x.reshape(b, c, h // 2, 2, w // 2, 2).max(axis=(3, 5))


def _upsample2_full(x):
    # bilinear x2, align_corners=True — gather-free parity decomposition:
    #   out[2m]   = x[m-1]*we[m] + x[m]*(1-we[m]),  we[m] = m/(2n-1)
    #   out[2m+1] = x[m]*(1-wo[m]) + x[m+1]*wo[m],  wo[m] = (n-1-m)/(2n-1)
    # (edge clamps are exact because we[0] = wo[n-1] = 0)
    import jax.numpy as jnp

    def up_axis(t, axis):
        t = jnp.moveaxis(t, axis, -1)
        n = t.shape[-1]
        we = (np.arange(n) / (2 * n - 1)).astype(np.float32)
        wo = ((n - 1 - np.arange(n)) / (2 * n - 1)).astype(np.float32)
        tm1 = jnp.concatenate([t[..., :1], t[..., :-1]], -1)
        tp1 = jnp.concatenate([t[..., 1:], t[..., -1:]], -1)
        out_e = tm1 * we + t * (1.0 - we)
        out_o = t * (1.0 - wo) + tp1 * wo
        out = jnp.stack([out_e, out_o], -1).reshape(t.shape[:-1] + (2 * n,))
        return jnp.moveaxis(out, -1, axis)

    return up_axis(up_axis(x, 2), 3)


def _block_sharded(x, extra, p, axis_name):
    import jax
    import jax.numpy as jnp
    a = jax.nn.relu(_axial_attention_sharded(x, p['attn'], axis_name))
    cat = jnp.concatenate([a, x] + extra, axis=1)
    y = jax.nn.relu(_conv1x1(cat, p['conv_w']))
    return _bn_sharded(y, p['bn_g'], p['bn_b'], axis_name)


def _block_local(x, extra, p):
    import jax
    import jax.numpy as jnp
    a = jax.nn.relu(_axial_attention_local(x, p['attn']))
    cat = jnp.concatenate([a, x] + extra, axis=1)
    y = jax.nn.relu(_conv1x1(cat, p['conv_w']))
    return _bn_local(y, p['bn_g'], p['bn_b'])


# ---------------- sharded per-block programs ------------------------------

def _build_sharded():
    import jax
    from jax.sharding import Mesh, NamedSharding, PartitionSpec as P
    from jax.experimental.shard_map import shard_map

    try:
        jax.config.update("jax_compilation_cache_dir", "/var/tmp/jax_cache")
        jax.config.update("jax_persistent_cache_min_entry_size_bytes", -1)
        jax.config.update("jax_persistent_cache_min_compile_time_secs", 2)
    except Exception:
        pass

    devices = jax.devices()[:NCORES]
    assert len(devices) >= NCORES
    mesh = Mesh(np.asarray(devices[:NCORES]), ('c',))
    xspec = P(None, None, 'c', None)

    def smap(f, nin):
        return jax.jit(shard_map(
            f, mesh=mesh,
            in_specs=tuple([xspec] * nin + [P()]),
            out_specs=xspec, check_rep=False))

    def enc_body(x, p):
        return _block_sharded(x, [], p, 'c')

    def down_body(x, p):
        return _block_sharded(_maxpool2(x), [], p, 'c')

    def up_body(x, res, p):
        import jax
        hl = x.shape[2]
        xf = jax.lax.all_gather(x, 'c', axis=2, tiled=True)
        up = _upsample2_full(xf)
        idx = jax.lax.axis_index('c')
        xu = jax.lax.dynamic_slice_in_dim(up, idx * 2 * hl, 2 * hl, axis=2)
        return _block_sharded(xu, [res], p, 'c')

    def dec_body(x, w):
        return _conv1x1(x, w)

    fns = {
        'enc': smap(enc_body, 1),
        'down': smap(down_body, 1),
        'up': smap(up_body, 2),
        'dec': smap(dec_body, 1),
    }
    return fns, mesh, NamedSharding(mesh, xspec), NamedSharding(mesh, P())


def _run_sharded(x, params):
    import jax
    if 'sharded' not in _CACHE:
        _CACHE['sharded'] = _build_sharded()
    fns, mesh, xsh, rsh = _CACHE['sharded']

    put = lambda a: jax.device_put(a, rsh)
    xd = jax.device_put(np.asarray(x, np.float32), xsh)
    pd = jax.tree.map(put, dict(params))

    t0 = time.time()
    x1 = fns['enc'](xd, pd['encode'])
    _log(f"encode done {time.time()-t0:.1f}s")
    skips = [x1]
    cur = x1
    for name in ('down1', 'down2', 'down3', 'down4'):
        t0 = time.time()
        cur = fns['down'](cur, pd[name])
        skips.append(cur)
        _log(f"{name} done {time.time()-t0:.1f}s")
    for name, skip in zip(('up1', 'up2', 'up3', 'up4'),
                          (skips[3], skips[2], skips[1], skips[0])):
        t0 = time.time()
        cur = fns['up'](cur, skip, pd[name])
        _log(f"{name} done {time.time()-t0:.1f}s")
    t0 = time.time()
    out = fns['dec'](cur, pd['decode_w'])
    _log(f"decode done {time.time()-t0:.1f}s")
    return np.asarray(out).astype(np.float32)


# ---------------- fallbacks ----------------------------------------------

def _run_single(x, params):
    import jax

    if 'single' not in _CACHE:
        def body(x, params):
            p = params
            x1 = _block_local(x, [], p['encode'])
            x2 = _block_local(_maxpool2(x1), [], p['down1'])
            x3 = _block_local(_maxpool2(x2), [], p['down2'])
            x4 = _block_local(_maxpool2(x3), [], p['down3'])
            x5 = _block_local(_maxpool2(x4), [], p['down4'])

            def up(t):
                return _upsample2_full(t)

            x6 = _block_local(up(x5), [x4], p['up1'])
            x7 = _block_local(up(x6), [x3], p['up2'])
            x8 = _block_local(up(x7), [x2], p['up3'])
            x9 = _block_local(up(x8), [x1], p['up4'])
            return _conv1x1(x9, p['decode_w'])

        _CACHE['single'] = jax.jit(body)
    return np.asarray(_CACHE['single'](np.asarray(x, np.float32),
                                       dict(params))).astype(np.float32)


def kernel(x, params):
    import jax

    x = np.asarray(x, dtype=np.float32)
    try:
        return _run_sharded(x, params)
    except Exception as e:
        _log(f"sharded path failed: {e!r}")
    try:
        return _run_single(x, params)
    except Exception as e:
        _log(f"single-device path failed: {e!r}")
    _CACHE.pop('single', None)
    with jax.default_device(jax.devices('cpu')[0]):
        out = _run_single(x, params)
    _CACHE.pop('single', None)
    return out
